# revision 7
# baseline (speedup 1.0000x reference)
"""Trainium2 Bass kernel for a GNN message-passing layer.

Reference computation (all fp32):
    messages = h[src] @ W_msg.T            # [E, D]
    agg      = segment_sum(messages, dst)  # [N, D]
    out      = relu(concat(h, agg) @ W_upd.T + b_upd)

Key algebraic restructure: segment_sum is linear, so
    agg = A @ W_msg.T          where A = segment_sum(h[src], dst)
and the update splits W_upd = [Wu1 | Wu2]:
    out.T = relu(Wu1 @ h.T + (Wu2 @ W_msg) @ A.T + b)
so the device only computes A (a pure gather + scatter-add) plus two small
fused matmuls.  Wc = Wu2 @ W_msg is precomputed on host.

Sharding: nodes are partitioned contiguously across the 8 cores by dst.
Each core processes exactly the edges whose dst lands in its node shard
(host buckets edges by 128-node dst block), so no collectives are needed.

The kernel is SWDGE-descriptor-generation bound: every gathered h row costs
one software-generated DMA descriptor (~2.2 ns, serialized on the GpSimd
engine ucode).  Gathers sized at one (block, parity) (~2.2k descriptors)
fit the SWDGE ring; larger instructions stall on ring reclaim (measured).

Per core, per destination-node block (128 nodes):
  - the block's edges are padded to a fixed number of 128-edge chunks;
    pad slots gather an all-zero row appended to h, so they contribute 0
  - two dma_gather instructions fetch h[src] (bf16) for the block's edges
    (indices are int16, so rows are split even/odd and gathered from
    strided views h[0::2] / h[1::2] with idx = src>>1), spread across 4
    SWDGE queues.  Edges are sorted by src within each bucket for DRAM
    page locality.
  - scatter-add via a 0/1 staircase: ONE VectorE tensor_tensor per block
        S[e, c, jj] = (jj < rel[e,c] + 0.5)        in {0, 1}, jj = 0..129
    batched over all chunks of the block.  relp is stored duplicated
    (each value twice) so every operand's innermost AP dim is
    (stride 1, count 2) 2-byte — this keeps the DVE in its 2x perf mode
    despite the broadcasts.  One TensorE matmul per chunk accumulates
        psum[i, jj] += sum_e g[e, i] * S[e, jj]     (bf16 x bf16 -> fp32)
    The node sums are adjacent-column differences
        A.T[i, b*128 + j] = psum[i, j] - psum[i, j+1]
Phase 2 (per 4-block group): the adjacent-column difference is taken on
VectorE (fp32 psum copies -> bf16), then two bf16 matmuls
    out.T = relu(Wu1 @ h.T + Wc @ diff + b)
run at full PE rate (fp32 matmuls cost 4 cycles/row; bf16 costs 1).
"""

import contextlib

import numpy as np

import concourse.bass as bass
import concourse.mybir as mybir
import concourse.tile as tile
from concourse import bacc
from concourse.bass_utils import run_bass_kernel_spmd

P = 128  # SBUF partitions
D = 128  # feature dim (in_dim == out_dim == 128)
N_CORES = 8
CHUNK = 128  # edges per matmul chunk
W129 = CHUNK + 1  # staircase width per block (psum / buf)
W130 = CHUNK + 2  # staircase width incl. pad col (even for 2x DVE mode)
GAT_BUFS = 5  # gather tile pool depth
WARM = GAT_BUFS  # first blocks gather pads for real (warm every pool buf)
SCRATCH = 16384  # SWDGE descriptor carveout bytes/partition (default 16384)

_prog_cache: dict = {}


def _build_program(
    N: int, SP: int, NB: int, KE: int, KO: int, nreg=None, loop_iters=None
):
    """One SPMD program, shared by all 8 cores.

    N      : rows of the (replicated) h table incl. 2 appended zero rows
    SP     : padded nodes per core (NB * 128)
    NB     : 128-node blocks per core
    KE, KO : 128-edge chunks per block for even-src / odd-src edges
    loop_iters : if set, wrap the compute body in a For_i hardware loop
                 executing it that many times (wall-clock timing harness)
    """
    f32 = mybir.dt.float32
    bf16 = mybir.dt.bfloat16
    i16 = mybir.dt.int16
    NCH = KE + KO
    BCOLS = NCH * 8  # idx int16 columns per block
    if nreg is None:
        nreg = [(KE * CHUNK, KO * CHUNK)] * NB

    nc = bacc.Bacc(
        "TRN2",
        target_bir_lowering=False,
        num_swdge_queues=4,
        dynamic_dma_scratch_size=SCRATCH,
    )

    h_d = nc.dram_tensor("h", [N, D], bf16, kind="ExternalInput")
    hsT_d = nc.dram_tensor("hsT", [P, SP], bf16, kind="ExternalInput")
    idx_d = nc.dram_tensor("idx", [P, NB * BCOLS], i16, kind="ExternalInput")
    relp_d = nc.dram_tensor("relp", [P, NB * NCH * 2], bf16, kind="ExternalInput")
    iota_d = nc.dram_tensor("iota", [P, W130], bf16, kind="ExternalInput")
    w1_d = nc.dram_tensor("w1T", [D, D], bf16, kind="ExternalInput")
    wc_d = nc.dram_tensor("wcT", [D, D], bf16, kind="ExternalInput")
    b_d = nc.dram_tensor("bias", [P, 1], f32, kind="ExternalInput")
    out_d = nc.dram_tensor("outT", [P, SP], f32, kind="ExternalOutput")

    h_even = h_d[0:N:2, :]
    h_odd = h_d[1:N:2, :]

    with tile.TileContext(nc) as tc:
        with (
            tc.tile_pool(name="constp", bufs=1) as constp,
            tc.tile_pool(name="gatp", bufs=5) as gatp,
            tc.tile_pool(name="sp_", bufs=3) as sp_,
            tc.tile_pool(name="aggp", bufs=1) as aggp,
            tc.tile_pool(name="diffp", bufs=2) as diffp,
            tc.tile_pool(name="outp", bufs=3) as outp,
            tc.tile_pool(name="psp", bufs=6, space="PSUM") as psp,
            tc.tile_pool(name="ps2p", bufs=2, space="PSUM") as ps2p,
        ):
            # load order matters: the first gathers wait on iota/idx/relp,
            # so those go first (idx split per block); hsT (phase 2) last
            iota_t = constp.tile([P, W130], bf16)
            nc.sync.dma_start(iota_t[:], iota_d[:])
            # idx loads split 3-way so the first gathers start immediately
            # (a monolithic load costs ~14us of startup; per-block splits
            # put a sem-wait on the serial GpSimd row per gather — worse)
            idx_t = constp.tile([P, NB * BCOLS], i16)
            nc.sync.dma_start(idx_t[:, 0:BCOLS], idx_d[:, 0:BCOLS])
            cut = min(5, NB) * BCOLS
            nc.sync.dma_start(idx_t[:, BCOLS:cut], idx_d[:, BCOLS:cut])
            if cut < NB * BCOLS:
                nc.sync.dma_start(idx_t[:, cut:], idx_d[:, cut:])
            relp_t = constp.tile([P, NB * NCH * 2], bf16)
            nc.sync.dma_start(relp_t[:], relp_d[:])
            w1_t = constp.tile([D, D], bf16)
            nc.sync.dma_start(w1_t[:], w1_d[:])
            wc_t = constp.tile([D, D], bf16)
            nc.sync.dma_start(wc_t[:], wc_d[:])
            b_t = constp.tile([P, 1], f32)
            nc.sync.dma_start(b_t[:], b_d[:])
            hsT_t = constp.tile([P, SP], bf16)
            nc.sync.dma_start(hsT_t[:], hsT_d[:])

            # staircase psum copies: per block 129 columns
            buf_t = aggp.tile([P, NB * W129], f32)

            iota_ab = iota_t[:].rearrange("p (a b) -> p a b", b=2)

            loop_cm = (
                tc.For_i(0, loop_iters, 1)
                if loop_iters is not None
                else contextlib.nullcontext()
            )
            with loop_cm:
                # Phase 1: staircase accumulation per block
                for b in range(NB):
                    g_t = gatp.tile([P, NCH * D], bf16)
                    g3 = g_t[:].rearrange("p (c d) -> p c d", c=NCH)
                    icol = b * BCOLS
                    nc.gpsimd.dma_gather(
                        out_ap=g3[:, 0:KE, :],
                        in_ap=h_even,
                        idxs_ap=idx_t[:, icol : icol + KE * 8],
                        num_idxs=KE * CHUNK,
                        num_idxs_reg=int(nreg[b][0]),
                        elem_size=D,
                        elem_step=2 * D,
                        single_packet=False,
                        queue_num=(2 * b) % 4,
                    )
                    nc.gpsimd.dma_gather(
                        out_ap=g3[:, KE:NCH, :],
                        in_ap=h_odd,
                        idxs_ap=idx_t[:, icol + KE * 8 : icol + BCOLS],
                        num_idxs=KO * CHUNK,
                        num_idxs_reg=int(nreg[b][1]),
                        elem_size=D,
                        elem_step=2 * D,
                        single_packet=False,
                        queue_num=(2 * b + 1) % 4,
                    )
                    # ONE DVE op: S[p, c, jj] = (iota[jj] < relp[p, c])
                    s_t = sp_.tile([P, NCH * W130], bf16)
                    relp_b = (
                        relp_t[:, b * NCH * 2 : (b + 1) * NCH * 2]
                        .rearrange("p (c t) -> p c t", t=2)
                        .unsqueeze(2)
                        .broadcast_to([P, NCH, W130 // 2, 2])
                    )
                    iota_b = iota_ab.unsqueeze(1).broadcast_to(
                        [P, NCH, W130 // 2, 2]
                    )
                    s_b = s_t[:].rearrange(
                        "p (c a b) -> p c a b", a=W130 // 2, b=2
                    )
                    nc.vector.tensor_tensor(
                        out=s_b, in0=iota_b, in1=relp_b, op=mybir.AluOpType.is_lt
                    )
                    s3 = s_t[:].rearrange("p (c w) -> p c w", w=W130)
                    ps_t = psp.tile([P, W129], f32)
                    for c in range(NCH):
                        nc.tensor.matmul(
                            out=ps_t[:],
                            lhsT=g_t[:, c * D : (c + 1) * D],
                            rhs=s3[:, c, 0:W129],
                            start=(c == 0),
                            stop=(c == NCH - 1),
                        )
                    nc.scalar.activation(
                        out=buf_t[:, b * W129 : (b + 1) * W129],
                        in_=ps_t[:],
                        func=mybir.ActivationFunctionType.Copy,
                    )

                # Phase 2 per 4-block group:
                #   diff = bufA - bufB (VectorE, fp32 -> bf16)
                #   out.T = relu(Wu1 @ h.T + Wc @ diff + b)   (bf16 matmuls)
                buf3 = buf_t[:].rearrange("p (b j) -> p b j", j=W129)
                b0 = 0
                while b0 < NB:
                    nb = min(4, NB - b0)
                    w = nb * CHUNK
                    col = b0 * CHUNK
                    d_t = diffp.tile([P, 512], bf16)
                    d3 = d_t[:].rearrange("p (b j) -> p b j", j=CHUNK)
                    nc.vector.tensor_tensor(
                        out=d3[:, 0:nb, :],
                        in0=buf3[:, b0 : b0 + nb, 0:CHUNK],
                        in1=buf3[:, b0 : b0 + nb, 1:W129],
                        op=mybir.AluOpType.subtract,
                    )
                    ps2_t = ps2p.tile([P, 512], f32)
                    nc.tensor.matmul(
                        out=ps2_t[:, :w],
                        lhsT=w1_t[:],
                        rhs=hsT_t[:, col : col + w],
                        start=True,
                        stop=False,
                    )
                    nc.tensor.matmul(
                        out=ps2_t[:, :w],
                        lhsT=wc_t[:],
                        rhs=d_t[:, :w],
                        start=False,
                        stop=True,
                    )
                    o_t = outp.tile([P, 512], f32)
                    nc.scalar.activation(
                        o_t[:, :w],
                        ps2_t[:, :w],
                        mybir.ActivationFunctionType.Relu,
                        bias=b_t[:],
                    )
                    nc.sync.dma_start(out_d[:, col : col + w], o_t[:, :w])
                    b0 += nb

    nc.compile()
    return nc


def _prep_inputs(h, edge_index, W_msg, W_upd, b_upd):
    """Host-side sharding: bucket edges by destination-node block, then
    split each block's edges by src parity for the int16 dma_gather.

    Blocks are assigned to (core, slot) by descending edge count: slot s
    holds ranks [8s, 8s+8) spread across the 8 cores, so one SPMD-shared
    num_idxs_reg per (slot, parity) (the max over its 8 blocks) is tight.
    Pad gather slots beyond that count carry idx=-1 and are SKIPPED by the
    SWDGE ucode (no DMA packet).  Slots < WARM instead gather pads for real
    so every gather-pool buffer holds finite bf16 data before any skipped
    (stale-data) tail can appear under a zero staircase row.
    """
    import ml_dtypes

    N0, d = h.shape
    assert d == D
    E = edge_index.shape[1]

    SP = -(-N0 // (N_CORES * P)) * P  # padded nodes per core
    NB = SP // P
    n_blocks_tot = N_CORES * NB

    src = np.ascontiguousarray(edge_index[0]).astype(np.int64)
    dst = np.ascontiguousarray(edge_index[1]).astype(np.int64)

    # h with two appended zero rows (one per parity) for pad-edge gathers
    N = N0 + 2
    hg = np.zeros((N, D), dtype=ml_dtypes.bfloat16)
    hg[:N0] = h.astype(ml_dtypes.bfloat16)
    pad_idx = N0 // 2  # row N0 (even) / N0+1 (odd), both zero

    # order edges by (dst block, src parity), then src for DRAM locality
    gblock = dst >> 7
    parity = src & 1
    order = np.lexsort((src, gblock * 2 + parity))
    gb_s = gblock[order]
    par_s = parity[order]
    idx_s = (src[order] >> 1).astype(np.int16)
    rel_s = (dst[order] & 127).astype(np.float32)

    cnt = np.bincount(gb_s * 2 + par_s, minlength=2 * n_blocks_tot).reshape(-1, 2)
    KE = max(1, -(-int(cnt[:, 0].max()) // CHUNK))
    KO = max(1, -(-int(cnt[:, 1].max()) // CHUNK))
    NCH = KE + KO

    # block -> (core, slot) assignment, largest blocks on warm slots
    ranks = np.argsort(-(cnt[:, 0] + cnt[:, 1]), kind="stable")
    assign = ranks.reshape(NB, N_CORES)  # assign[s, c] = global block
    # shared per-(slot, parity) valid-index count, multiple of 16
    nregE = cnt[assign, 0].max(axis=1)
    nregO = cnt[assign, 1].max(axis=1)
    nregE = np.minimum(-(-np.maximum(nregE, 16) // 16) * 16, KE * CHUNK)
    nregO = np.minimum(-(-np.maximum(nregO, 16) // 16) * 16, KO * CHUNK)
    nregE[:WARM] = KE * CHUNK
    nregO[:WARM] = KO * CHUNK
    # per-block valid counts (slot of block): blocks pad with pad_idx up
    # to the slot's nreg, and -1 (skipped) beyond
    slot_of = np.empty(n_blocks_tot, dtype=np.int64)
    slot_of[assign.reshape(-1)] = np.repeat(np.arange(NB), N_CORES)
    nvalE = nregE[slot_of]
    nvalO = nregO[slot_of]

    capE, capO = KE * CHUNK, KO * CHUNK
    starts = np.zeros(2 * n_blocks_tot + 1, dtype=np.int64)
    np.cumsum(cnt.reshape(-1), out=starts[1:])
    pos = np.arange(E, dtype=np.int64) - starts[gb_s * 2 + par_s]

    colE = np.arange(capE)[None, :]
    colO = np.arange(capO)[None, :]
    idx_pad = np.empty((n_blocks_tot, NCH * CHUNK), dtype=np.int16)
    idx_pad[:, :capE] = np.where(colE < nvalE[:, None], pad_idx, -1)
    idx_pad[:, capE:] = np.where(colO < nvalO[:, None], pad_idx, -1)
    rel_pad = np.full((n_blocks_tot, NCH * CHUNK), -1.0, dtype=np.float32)
    slot = pos + par_s * capE
    idx_pad[gb_s, slot] = idx_s
    rel_pad[gb_s, slot] = rel_s

    # idx: wrapped [16, n/16] per (block, parity), replicated to all 8
    # partition groups (each gpsimd Q7 core reads its own group of 16)
    idx16 = np.empty((n_blocks_tot, 16, NCH * 8), dtype=np.int16)
    idx16[:, :, : KE * 8] = (
        idx_pad[:, :capE].reshape(n_blocks_tot, capE // 16, 16).transpose(0, 2, 1)
    )
    idx16[:, :, KE * 8 :] = (
        idx_pad[:, capE:].reshape(n_blocks_tot, capO // 16, 16).transpose(0, 2, 1)
    )
    idx_in = np.tile(idx16, (1, 8, 1))

    # relp = rel + 0.5 (staircase threshold), position-major [128, NCH],
    # each value DUPLICATED along the last axis (keeps DVE in 2x mode)
    relp_in = (rel_pad + 0.5).reshape(n_blocks_tot, NCH, CHUNK).transpose(0, 2, 1)
    relp_in = np.repeat(relp_in, 2, axis=2).astype(ml_dtypes.bfloat16)

    w1T = np.ascontiguousarray(
        W_upd[:, :D].T.astype(np.float32).astype(ml_dtypes.bfloat16)
    )
    wc = (W_upd[:, D:].astype(np.float64) @ W_msg.astype(np.float64)).astype(
        np.float32
    )
    wcT = np.ascontiguousarray(wc.T.astype(ml_dtypes.bfloat16))
    bias = np.ascontiguousarray(b_upd.astype(np.float32).reshape(P, 1))
    iota = np.ascontiguousarray(
        np.tile(np.arange(W130, dtype=np.float32), (P, 1)).astype(ml_dtypes.bfloat16)
    )

    hbf = h.astype(ml_dtypes.bfloat16)
    in_maps = []
    for c in range(N_CORES):
        blocks = assign[:, c]  # global block id per slot
        # hsT: node features for this core's assigned blocks, slot order
        hs = np.zeros((NB, P, D), dtype=ml_dtypes.bfloat16)
        for s, gb in enumerate(blocks):
            lo = gb * P
            hi = min(lo + P, N0)
            if hi > lo:
                hs[s, : hi - lo] = hbf[lo:hi]
        in_maps.append(
            {
                "h": hg,
                "hsT": np.ascontiguousarray(hs.reshape(SP, D).T),
                "idx": np.ascontiguousarray(
                    idx_in[blocks].transpose(1, 0, 2).reshape(P, NB * NCH * 8)
                ),
                "relp": np.ascontiguousarray(
                    relp_in[blocks].transpose(1, 0, 2).reshape(P, NB * NCH * 2)
                ),
                "iota": iota,
                "w1T": w1T,
                "wcT": wcT,
                "bias": bias,
            }
        )
    nreg = list(zip(nregE.tolist(), nregO.tolist()))
    return in_maps, N, SP, NB, KE, KO, nreg, assign


def kernel_with_results(h, edge_index, W_msg, W_upd, b_upd, loop_iters=None, **run_kwargs):
    in_maps, N, SP, NB, KE, KO, nreg, assign = _prep_inputs(
        h, edge_index, W_msg, W_upd, b_upd
    )

    key = (N, SP, NB, KE, KO, tuple(nreg), loop_iters)
    if key not in _prog_cache:
        _prog_cache[key] = _build_program(
            N, SP, NB, KE, KO, nreg=nreg, loop_iters=loop_iters
        )
    nc = _prog_cache[key]

    res = run_bass_kernel_spmd(nc, in_maps, core_ids=list(range(N_CORES)), **run_kwargs)

    N0 = N - 2
    out = np.empty((N0, D), dtype=np.float32)
    for c in range(N_CORES):
        oT = res.results[c]["outT"]  # [P, SP] fp32, slot-major columns
        for s in range(NB):
            gb = int(assign[s, c])
            lo = gb * P
            hi = min(lo + P, N0)
            if hi > lo:
                out[lo:hi] = oT[:, s * P : s * P + (hi - lo)].T
    return out, res


def kernel(h, edge_index, W_msg, W_upd, b_upd):
    out, _ = kernel_with_results(h, edge_index, W_msg, W_upd, b_upd)
    return out



# revision 10
# speedup vs baseline: 1.0210x; 1.0210x over previous
"""Trainium2 Bass kernel for a GNN message-passing layer.

Reference computation (all fp32):
    messages = h[src] @ W_msg.T            # [E, D]
    agg      = segment_sum(messages, dst)  # [N, D]
    out      = relu(concat(h, agg) @ W_upd.T + b_upd)

Key algebraic restructure: segment_sum is linear, so
    agg = A @ W_msg.T          where A = segment_sum(h[src], dst)
and the update splits W_upd = [Wu1 | Wu2]:
    out.T = relu(Wu1 @ h.T + (Wu2 @ W_msg) @ A.T + b)
so the device only computes A (a pure gather + scatter-add) plus two small
fused matmuls.  Wc = Wu2 @ W_msg is precomputed on host.

Sharding: nodes are partitioned contiguously across the 8 cores by dst.
Each core processes exactly the edges whose dst lands in its node shard
(host buckets edges by 128-node dst block), so no collectives are needed.

The kernel is SWDGE-descriptor-generation bound: every gathered h row costs
one software-generated DMA descriptor (~2.2 ns, serialized on the GpSimd
engine ucode).  Gathers sized at one (block, parity) (~2.2k descriptors)
fit the SWDGE ring; larger instructions stall on ring reclaim (measured).

Per core, per destination-node block (128 nodes):
  - the block's edges are padded to a fixed number of 128-edge chunks;
    pad slots gather an all-zero row appended to h, so they contribute 0
  - two dma_gather instructions fetch h[src] (bf16) for the block's edges
    (indices are int16, so rows are split even/odd and gathered from
    strided views h[0::2] / h[1::2] with idx = src>>1), spread across 4
    SWDGE queues.  Edges are sorted by src within each bucket for DRAM
    page locality.
  - scatter-add via a 0/1 staircase: ONE VectorE tensor_tensor per block
        S[e, c, jj] = (jj < rel[e,c] + 0.5)        in {0, 1}, jj = 0..129
    batched over all chunks of the block.  relp is stored duplicated
    (each value twice) so every operand's innermost AP dim is
    (stride 1, count 2) 2-byte — this keeps the DVE in its 2x perf mode
    despite the broadcasts.  One TensorE matmul per chunk accumulates
        psum[i, jj] += sum_e g[e, i] * S[e, jj]     (bf16 x bf16 -> fp32)
    The node sums are adjacent-column differences
        A.T[i, b*128 + j] = psum[i, j] - psum[i, j+1]
Phase 2 (per 4-block group): the adjacent-column difference is taken on
VectorE (fp32 psum copies -> bf16), then two bf16 matmuls
    out.T = relu(Wu1 @ h.T + Wc @ diff + b)
run at full PE rate (fp32 matmuls cost 4 cycles/row; bf16 costs 1).
"""

import contextlib

import numpy as np

import concourse.bass as bass
import concourse.mybir as mybir
import concourse.tile as tile
from concourse import bacc
from concourse.bass_utils import run_bass_kernel_spmd

P = 128  # SBUF partitions
D = 128  # feature dim (in_dim == out_dim == 128)
N_CORES = 8
CHUNK = 128  # edges per matmul chunk
W129 = CHUNK + 1  # staircase width per block (psum / buf)
W130 = CHUNK + 2  # staircase width incl. pad col (even for 2x DVE mode)
GAT_BUFS = 5  # gather tile pool depth
WARM = GAT_BUFS  # first blocks gather pads for real (warm every pool buf)
SCRATCH = 16384  # SWDGE descriptor carveout bytes/partition (default 16384)

_prog_cache: dict = {}


def _build_program(
    N: int, SP: int, NB: int, KE: int, KO: int, nreg=None, loop_iters=None
):
    """One SPMD program, shared by all 8 cores.

    N      : rows of the (replicated) h table incl. 2 appended zero rows
    SP     : padded nodes per core (NB * 128)
    NB     : 128-node blocks per core
    KE, KO : 128-edge chunks per block for even-src / odd-src edges
    loop_iters : if set, wrap the compute body in a For_i hardware loop
                 executing it that many times (wall-clock timing harness)
    """
    f32 = mybir.dt.float32
    bf16 = mybir.dt.bfloat16
    i16 = mybir.dt.int16
    NCH = KE + KO
    BCOLS = NCH * 8  # idx int16 columns per block
    if nreg is None:
        nreg = [(KE * CHUNK, KO * CHUNK)] * NB

    nc = bacc.Bacc(
        "TRN2",
        target_bir_lowering=False,
        num_swdge_queues=4,
        dynamic_dma_scratch_size=SCRATCH,
    )

    h_d = nc.dram_tensor("h", [N, D], bf16, kind="ExternalInput")
    hsT_d = nc.dram_tensor("hsT", [P, SP], bf16, kind="ExternalInput")
    idx_d = nc.dram_tensor("idx", [P, NB * BCOLS], i16, kind="ExternalInput")
    relp_d = nc.dram_tensor("relp", [P, NB * NCH * 2], bf16, kind="ExternalInput")
    iota_d = nc.dram_tensor("iota", [P, W130], bf16, kind="ExternalInput")
    w1_d = nc.dram_tensor("w1T", [D, D], bf16, kind="ExternalInput")
    wc_d = nc.dram_tensor("wcT", [D, D], bf16, kind="ExternalInput")
    b_d = nc.dram_tensor("bias", [P, 1], f32, kind="ExternalInput")
    out_d = nc.dram_tensor("outT", [P, SP], f32, kind="ExternalOutput")

    h_even = h_d[0:N:2, :]
    h_odd = h_d[1:N:2, :]

    with tile.TileContext(nc) as tc:
        with (
            tc.tile_pool(name="constp", bufs=1) as constp,
            tc.tile_pool(name="gatp", bufs=5) as gatp,
            tc.tile_pool(name="sp_", bufs=3) as sp_,
            tc.tile_pool(name="aggp", bufs=1) as aggp,
            tc.tile_pool(name="diffp", bufs=2) as diffp,
            tc.tile_pool(name="outp", bufs=3) as outp,
            tc.tile_pool(name="psp", bufs=6, space="PSUM") as psp,
            tc.tile_pool(name="ps2p", bufs=2, space="PSUM") as ps2p,
        ):
            # load order matters: the first gathers wait on iota/idx/relp,
            # so those go first (idx split per block); hsT (phase 2) last
            iota_t = constp.tile([P, W130], bf16)
            nc.sync.dma_start(iota_t[:], iota_d[:])
            # idx loads split 3-way so the first gathers start immediately
            # (a monolithic load costs ~14us of startup; per-block splits
            # put a sem-wait on the serial GpSimd row per gather — worse)
            idx_t = constp.tile([P, NB * BCOLS], i16)
            nc.sync.dma_start(idx_t[:, 0:BCOLS], idx_d[:, 0:BCOLS])
            cut = min(5, NB) * BCOLS
            nc.sync.dma_start(idx_t[:, BCOLS:cut], idx_d[:, BCOLS:cut])
            if cut < NB * BCOLS:
                nc.sync.dma_start(idx_t[:, cut:], idx_d[:, cut:])
            relp_t = constp.tile([P, NB * NCH * 2], bf16)
            nc.sync.dma_start(relp_t[:], relp_d[:])
            w1_t = constp.tile([D, D], bf16)
            nc.sync.dma_start(w1_t[:], w1_d[:])
            wc_t = constp.tile([D, D], bf16)
            nc.sync.dma_start(wc_t[:], wc_d[:])
            b_t = constp.tile([P, 1], f32)
            nc.sync.dma_start(b_t[:], b_d[:])
            hsT_t = constp.tile([P, SP], bf16)
            nc.sync.dma_start(hsT_t[:], hsT_d[:])

            # staircase psum copies: per block 129 columns
            buf_t = aggp.tile([P, NB * W129], f32)

            iota_ab = iota_t[:].rearrange("p (a b) -> p a b", b=2)

            loop_cm = (
                tc.For_i(0, loop_iters, 1)
                if loop_iters is not None
                else contextlib.nullcontext()
            )
            with loop_cm:
                # Phase 1: staircase accumulation per block
                for b in range(NB):
                    g_t = gatp.tile([P, NCH * D], bf16)
                    g3 = g_t[:].rearrange("p (c d) -> p c d", c=NCH)
                    icol = b * BCOLS
                    nc.gpsimd.dma_gather(
                        out_ap=g3[:, 0:KE, :],
                        in_ap=h_even,
                        idxs_ap=idx_t[:, icol : icol + KE * 8],
                        num_idxs=KE * CHUNK,
                        num_idxs_reg=KE * CHUNK,
                        elem_size=D,
                        elem_step=2 * D,
                        single_packet=False,
                        queue_num=(2 * b) % 4,
                    )
                    nc.gpsimd.dma_gather(
                        out_ap=g3[:, KE:NCH, :],
                        in_ap=h_odd,
                        idxs_ap=idx_t[:, icol + KE * 8 : icol + BCOLS],
                        num_idxs=KO * CHUNK,
                        num_idxs_reg=KO * CHUNK,
                        elem_size=D,
                        elem_step=2 * D,
                        single_packet=False,
                        queue_num=(2 * b + 1) % 4,
                    )
                    # ONE DVE op: S[p, c, jj] = (iota[jj] < relp[p, c])
                    s_t = sp_.tile([P, NCH * W130], bf16)
                    relp_b = (
                        relp_t[:, b * NCH * 2 : (b + 1) * NCH * 2]
                        .rearrange("p (c t) -> p c t", t=2)
                        .unsqueeze(2)
                        .broadcast_to([P, NCH, W130 // 2, 2])
                    )
                    iota_b = iota_ab.unsqueeze(1).broadcast_to(
                        [P, NCH, W130 // 2, 2]
                    )
                    s_b = s_t[:].rearrange(
                        "p (c a b) -> p c a b", a=W130 // 2, b=2
                    )
                    nc.vector.tensor_tensor(
                        out=s_b, in0=iota_b, in1=relp_b, op=mybir.AluOpType.is_lt
                    )
                    s3 = s_t[:].rearrange("p (c w) -> p c w", w=W130)
                    ps_t = psp.tile([P, W129], f32)
                    for c in range(NCH):
                        nc.tensor.matmul(
                            out=ps_t[:],
                            lhsT=g_t[:, c * D : (c + 1) * D],
                            rhs=s3[:, c, 0:W129],
                            start=(c == 0),
                            stop=(c == NCH - 1),
                        )
                    nc.scalar.activation(
                        out=buf_t[:, b * W129 : (b + 1) * W129],
                        in_=ps_t[:],
                        func=mybir.ActivationFunctionType.Copy,
                    )

                # Phase 2 per 4-block group:
                #   diff = bufA - bufB (VectorE, fp32 -> bf16)
                #   out.T = relu(Wu1 @ h.T + Wc @ diff + b)   (bf16 matmuls)
                buf3 = buf_t[:].rearrange("p (b j) -> p b j", j=W129)
                b0 = 0
                while b0 < NB:
                    nb = min(4, NB - b0)
                    w = nb * CHUNK
                    col = b0 * CHUNK
                    d_t = diffp.tile([P, 512], bf16)
                    d3 = d_t[:].rearrange("p (b j) -> p b j", j=CHUNK)
                    nc.vector.tensor_tensor(
                        out=d3[:, 0:nb, :],
                        in0=buf3[:, b0 : b0 + nb, 0:CHUNK],
                        in1=buf3[:, b0 : b0 + nb, 1:W129],
                        op=mybir.AluOpType.subtract,
                    )
                    ps2_t = ps2p.tile([P, 512], f32)
                    nc.tensor.matmul(
                        out=ps2_t[:, :w],
                        lhsT=w1_t[:],
                        rhs=hsT_t[:, col : col + w],
                        start=True,
                        stop=False,
                    )
                    nc.tensor.matmul(
                        out=ps2_t[:, :w],
                        lhsT=wc_t[:],
                        rhs=d_t[:, :w],
                        start=False,
                        stop=True,
                    )
                    o_t = outp.tile([P, 512], f32)
                    nc.scalar.activation(
                        o_t[:, :w],
                        ps2_t[:, :w],
                        mybir.ActivationFunctionType.Relu,
                        bias=b_t[:],
                    )
                    nc.sync.dma_start(out_d[:, col : col + w], o_t[:, :w])
                    b0 += nb

    nc.compile()
    return nc


def _prep_inputs(h, edge_index, W_msg, W_upd, b_upd):
    """Host-side sharding: bucket edges by destination-node block, then
    split each block's edges by src parity for the int16 dma_gather.

    Blocks are assigned to (core, slot) by descending edge count: slot s
    holds ranks [8s, 8s+8) spread across the 8 cores, so one SPMD-shared
    num_idxs_reg per (slot, parity) (the max over its 8 blocks) is tight.
    Pad gather slots beyond that count carry idx=-1 and are SKIPPED by the
    SWDGE ucode (no DMA packet).  Slots < WARM instead gather pads for real
    so every gather-pool buffer holds finite bf16 data before any skipped
    (stale-data) tail can appear under a zero staircase row.
    """
    import ml_dtypes

    N0, d = h.shape
    assert d == D
    E = edge_index.shape[1]

    SP = -(-N0 // (N_CORES * P)) * P  # padded nodes per core
    NB = SP // P
    n_blocks_tot = N_CORES * NB

    src = np.ascontiguousarray(edge_index[0]).astype(np.int64)
    dst = np.ascontiguousarray(edge_index[1]).astype(np.int64)

    # h with two appended zero rows (one per parity) for pad-edge gathers
    N = N0 + 2
    hg = np.zeros((N, D), dtype=ml_dtypes.bfloat16)
    hg[:N0] = h.astype(ml_dtypes.bfloat16)
    pad_idx = N0 // 2  # row N0 (even) / N0+1 (odd), both zero

    # order edges by (dst block, src parity), then src for DRAM locality
    gblock = dst >> 7
    parity = src & 1
    order = np.lexsort((src, gblock * 2 + parity))
    gb_s = gblock[order]
    par_s = parity[order]
    idx_s = (src[order] >> 1).astype(np.int16)
    rel_s = (dst[order] & 127).astype(np.float32)

    cnt = np.bincount(gb_s * 2 + par_s, minlength=2 * n_blocks_tot).reshape(-1, 2)
    KE = max(1, -(-int(cnt[:, 0].max()) // CHUNK))
    KO = max(1, -(-int(cnt[:, 1].max()) // CHUNK))
    NCH = KE + KO

    # block -> (core, slot) assignment, largest blocks on warm slots
    ranks = np.argsort(-(cnt[:, 0] + cnt[:, 1]), kind="stable")
    assign = ranks.reshape(NB, N_CORES)  # assign[s, c] = global block
    nregE = np.full(NB, KE * CHUNK, dtype=np.int64)
    nregO = np.full(NB, KO * CHUNK, dtype=np.int64)

    capE, capO = KE * CHUNK, KO * CHUNK
    starts = np.zeros(2 * n_blocks_tot + 1, dtype=np.int64)
    np.cumsum(cnt.reshape(-1), out=starts[1:])
    pos = np.arange(E, dtype=np.int64) - starts[gb_s * 2 + par_s]

    idx_pad = np.full((n_blocks_tot, NCH * CHUNK), pad_idx, dtype=np.int16)
    rel_pad = np.full((n_blocks_tot, NCH * CHUNK), -1.0, dtype=np.float32)
    slot = pos + par_s * capE
    idx_pad[gb_s, slot] = idx_s
    rel_pad[gb_s, slot] = rel_s

    # idx: wrapped [16, n/16] per (block, parity), replicated to all 8
    # partition groups (each gpsimd Q7 core reads its own group of 16)
    idx16 = np.empty((n_blocks_tot, 16, NCH * 8), dtype=np.int16)
    idx16[:, :, : KE * 8] = (
        idx_pad[:, :capE].reshape(n_blocks_tot, capE // 16, 16).transpose(0, 2, 1)
    )
    idx16[:, :, KE * 8 :] = (
        idx_pad[:, capE:].reshape(n_blocks_tot, capO // 16, 16).transpose(0, 2, 1)
    )
    idx_in = np.tile(idx16, (1, 8, 1))

    # relp = rel + 0.5 (staircase threshold), position-major [128, NCH],
    # each value DUPLICATED along the last axis (keeps DVE in 2x mode)
    relp_in = (rel_pad + 0.5).reshape(n_blocks_tot, NCH, CHUNK).transpose(0, 2, 1)
    relp_in = np.repeat(relp_in, 2, axis=2).astype(ml_dtypes.bfloat16)

    w1T = np.ascontiguousarray(
        W_upd[:, :D].T.astype(np.float32).astype(ml_dtypes.bfloat16)
    )
    wc = (W_upd[:, D:].astype(np.float64) @ W_msg.astype(np.float64)).astype(
        np.float32
    )
    wcT = np.ascontiguousarray(wc.T.astype(ml_dtypes.bfloat16))
    bias = np.ascontiguousarray(b_upd.astype(np.float32).reshape(P, 1))
    iota = np.ascontiguousarray(
        np.tile(np.arange(W130, dtype=np.float32), (P, 1)).astype(ml_dtypes.bfloat16)
    )

    hbf = h.astype(ml_dtypes.bfloat16)
    in_maps = []
    for c in range(N_CORES):
        blocks = assign[:, c]  # global block id per slot
        # hsT: node features for this core's assigned blocks, slot order
        hs = np.zeros((NB, P, D), dtype=ml_dtypes.bfloat16)
        for s, gb in enumerate(blocks):
            lo = gb * P
            hi = min(lo + P, N0)
            if hi > lo:
                hs[s, : hi - lo] = hbf[lo:hi]
        in_maps.append(
            {
                "h": hg,
                "hsT": np.ascontiguousarray(hs.reshape(SP, D).T),
                "idx": np.ascontiguousarray(
                    idx_in[blocks].transpose(1, 0, 2).reshape(P, NB * NCH * 8)
                ),
                "relp": np.ascontiguousarray(
                    relp_in[blocks].transpose(1, 0, 2).reshape(P, NB * NCH * 2)
                ),
                "iota": iota,
                "w1T": w1T,
                "wcT": wcT,
                "bias": bias,
            }
        )
    nreg = list(zip(nregE.tolist(), nregO.tolist()))
    return in_maps, N, SP, NB, KE, KO, nreg, assign


def kernel_with_results(h, edge_index, W_msg, W_upd, b_upd, loop_iters=None, **run_kwargs):
    in_maps, N, SP, NB, KE, KO, nreg, assign = _prep_inputs(
        h, edge_index, W_msg, W_upd, b_upd
    )

    key = (N, SP, NB, KE, KO, tuple(nreg), loop_iters)
    if key not in _prog_cache:
        _prog_cache[key] = _build_program(
            N, SP, NB, KE, KO, nreg=nreg, loop_iters=loop_iters
        )
    nc = _prog_cache[key]

    res = run_bass_kernel_spmd(nc, in_maps, core_ids=list(range(N_CORES)), **run_kwargs)

    N0 = N - 2
    out = np.empty((N0, D), dtype=np.float32)
    for c in range(N_CORES):
        oT = res.results[c]["outT"]  # [P, SP] fp32, slot-major columns
        for s in range(NB):
            gb = int(assign[s, c])
            lo = gb * P
            hi = min(lo + P, N0)
            if hi > lo:
                out[lo:hi] = oT[:, s * P : s * P + (hi - lo)].T
    return out, res


def kernel(h, edge_index, W_msg, W_upd, b_upd):
    out, _ = kernel_with_results(h, edge_index, W_msg, W_upd, b_upd)
    return out

